# revision 11
# baseline (speedup 1.0000x reference)
"""MoE routing kernel for Trainium2 (8 NeuronCores, batch-parallel).

Per batch element b (one NeuronCore each):
    pooled = mean_s x[b]; h = tanh(pooled @ rw1 + rb1)
    logits = h @ rw2 + rb2; probs = softmax(logits)
    top-3 of 4 experts, renormalized: w[e] = probs[e]*(probs[e]>min)/(1-min)
    hid_e = gelu_tanh(x[b] @ zw1[e] + zb1[e]); z_e = hid_e @ zw2[e] + zb2[e]
    out[b] = x[b] + sum_e w[e] * z_e

Only the 3 active experts are computed: the router (fp16-pooled, fp32
chain) picks the dropped expert; three static "slots" then stream that
expert's weights with register-indexed (runtime-offset) DMAs.

Precision: GEMM2 and the first DSPLIT columns of GEMM1 run in fp8 e4m3
with DoubleRow perf mode (2x PE rate); the remaining BF_DC d-chunks of
GEMM1 run in fp16 (kills x/zw1 quantization error on that slice),
landing rel err ~1.8e-2 vs the 2e-2 gate. Router, psum accumulation,
residual add and combine all stay fp32.
"""
import sys

sys.path.insert(0, "/opt/trn_rl_repo")

import numpy as np
import ml_dtypes

import concourse.bacc as bacc
import concourse.bass as bass
import concourse.mybir as mybir
import concourse.tile as tile
import concourse.bass_isa as bass_isa
from concourse.bass_utils import run_bass_kernel_spmd
from concourse.masks import make_identity

S, D, F, E, H = 512, 1024, 4096, 4, 256
K = 3            # active experts (top-3 of 4)
P = 128
TC = S // P      # 4 token chunks
DC = D // P      # 8 d chunks
FC = F // P      # 32 ff chunks
FP2 = FC // 2    # 16 ff pairs for GEMM2 DoubleRow
NH = D // 512    # 2 output d halves

BF_DC = 4              # d-chunks of GEMM1 upgraded to fp16 (accuracy knob)
DRP = (DC - BF_DC) // 2  # fp8 DoubleRow d-pairs in GEMM1
DSPLIT = DRP * 256

F32 = mybir.dt.float32
F16 = mybir.dt.float16
I32 = mybir.dt.int32
FP8 = mybir.dt.float8e4
DR = mybir.MatmulPerfMode.DoubleRow
GELU = mybir.ActivationFunctionType.Gelu_apprx_tanh
COPY = mybir.ActivationFunctionType.Copy

SX = 8.0         # x fp8 quant scale
SW1 = 64.0       # zw1 fp8 quant scale (fp16 part pre-scaled by SX*SW1)
SW2 = 128.0      # zw2 fp8 quant scale


def build_nc():
    nc = bacc.Bacc("TRN2", target_bir_lowering=False, debug=False)

    x_d = nc.dram_tensor("x", [S, D], F32, kind="ExternalInput")
    rw1_d = nc.dram_tensor("rw1", [D, H], F32, kind="ExternalInput")
    rb1_d = nc.dram_tensor("rb1", [H], F32, kind="ExternalInput")
    rw2_d = nc.dram_tensor("rw2", [H, E], F32, kind="ExternalInput")
    rb2_d = nc.dram_tensor("rb2", [E], F32, kind="ExternalInput")
    if DRP:
        zw1q_d = nc.dram_tensor("zw1q", [E * P, FC, DRP, 2, P], FP8,
                                kind="ExternalInput")
    if BF_DC:
        zw1h_d = nc.dram_tensor("zw1h", [E * P, FC, BF_DC, P], F16,
                                kind="ExternalInput")
    zb1_d = nc.dram_tensor("zb1r", [E * P, FC], F32, kind="ExternalInput")
    zw2_d = nc.dram_tensor("zw2q", [E * P, FP2, 2, D], FP8, kind="ExternalInput")
    zb2_d = nc.dram_tensor("zb2", [E, D], F32, kind="ExternalInput")
    out_d = nc.dram_tensor("out", [S, D], F32, kind="ExternalOutput")

    with tile.TileContext(nc) as tc:
        with (
            tc.tile_pool(name="const", bufs=1) as const,
            tc.tile_pool(name="xb", bufs=1) as xb,
            tc.tile_pool(name="wstream", bufs=1) as wstream,
            tc.tile_pool(name="ps", bufs=8, space="PSUM") as ps,
        ):
            ident = const.tile([P, P], F16, name="ident")
            make_identity(nc, ident)
            ones16 = const.tile([P, 1], F16, name="ones16")
            nc.vector.memset(ones16, 1.0)

            # x as 8 eighth DMAs alternating HWDGE rings (finer chunks land
            # the last byte ~3us earlier, shrinking the router gate)
            x_all = xb.tile([P, TC, D], F32, name="x_all")
            xv = x_d.rearrange("(t p) d -> p t d", p=P)
            for t in range(TC):
                for hh in range(2):
                    eng = nc.sync if (t * 2 + hh) % 2 == 0 else nc.scalar
                    eng.dma_start(
                        out=x_all[:, t:t + 1, hh * 512:(hh + 1) * 512],
                        in_=xv[:, t:t + 1, hh * 512:(hh + 1) * 512])
            x_sb = [x_all[:, t, :] for t in range(TC)]

            # small router weights on the scalar ring
            rw1_sb = const.tile([P, DC, H], F32, name="rw1_sb")
            nc.scalar.dma_start(out=rw1_sb, in_=rw1_d.rearrange("(c p) h -> p c h", p=P))
            rb1_row = const.tile([1, H], F32, name="rb1_row")
            nc.scalar.dma_start(out=rb1_row, in_=rb1_d.rearrange("(o h) -> o h", o=1))
            rw2T_sb = const.tile([1, E, H], F32, name="rw2T_sb")
            nc.scalar.dma_start(out=rw2T_sb, in_=rw2_d.rearrange("(o h) e -> o e h", o=1))
            rb2_sb = const.tile([1, E], F32, name="rb2_sb")
            nc.scalar.dma_start(out=rb2_sb, in_=rb2_d.rearrange("(o e) -> o e", o=1))
            zb2_sb = const.tile([1, E, D], F32, name="zb2_sb")
            nc.scalar.dma_start(out=zb2_sb, in_=zb2_d.rearrange("(o e) d -> o e d", o=1))

            # fp16 copy of x: feeds the router pooling AND the transposes
            x_f16 = xb.tile([P, TC, D], F16, name="x_f16")
            for t in range(TC):
                nc.vector.tensor_copy(out=x_f16[:, t, :], in_=x_sb[t])

            # ---------- router: fp16-exact pooled, fp32 chain ----------
            prow_ps = []
            for nh in range(NH):
                pr = ps.tile([1, 512], F32, name=f"prow{nh}", tag="ps")
                for t in range(TC):
                    nc.tensor.matmul(pr, ones16, x_f16[:, t, nh * 512:(nh + 1) * 512],
                                     start=(t == 0), stop=(t == TC - 1))
                prow_ps.append(pr)
            pooled_row = const.tile([1, D], F32, name="pooled_row")
            for nh in range(NH):
                nc.vector.tensor_copy(out=pooled_row[:, nh * 512:(nh + 1) * 512],
                                      in_=prow_ps[nh])
            pooled_dram = nc.dram_tensor("pooled_bounce", [1, D], F32)
            nc.sync.dma_start(out=pooled_dram[:, :], in_=pooled_row)
            pooled3 = const.tile([P, 1, DC], F32, name="pooled3")
            nc.sync.dma_start(out=pooled3,
                              in_=pooled_dram.rearrange("o (c p) -> p o c", p=P))
            pooled = pooled3[:, 0, :]

            # ---------- transposes: x_f16 -> fp8 DR pairs + fp16 columns ----------
            xT8 = [xb.tile([P, 2, S], FP8, name=f"xT8_{dp}") for dp in range(DRP)]
            xT16 = [xb.tile([P, S], F16, name=f"xT16_{j}") for j in range(BF_DC)]

            def emit_transposes(trange):
                for t in trange:
                    for dc in range(DC):
                        ptr = ps.tile([P, P], F16, name=f"ptr{t}_{dc}", tag="ps")
                        nc.tensor.transpose(ptr, x_f16[:, t, dc * P:(dc + 1) * P],
                                            ident)
                        if dc < 2 * DRP:
                            dp, i = dc // 2, dc % 2
                            nc.scalar.activation(
                                out=xT8[dp][:, i, t * P:(t + 1) * P], in_=ptr,
                                func=COPY, scale=SX)
                        else:
                            nc.scalar.activation(
                                out=xT16[dc - 2 * DRP][:, t * P:(t + 1) * P],
                                in_=ptr, func=COPY)

            emit_transposes([0, 1])

            # h = tanh(pooled @ rw1 / S + rb1): per-partition partials on DVE,
            # cross-partition sum via ones matmul
            hacc = const.tile([P, H], F32, name="hacc")
            nc.vector.tensor_scalar(out=hacc, in0=rw1_sb[:, 0, :],
                                    scalar1=pooled[:, 0:1], scalar2=None,
                                    op0=mybir.AluOpType.mult)
            for dc in range(1, DC):
                nc.vector.scalar_tensor_tensor(out=hacc, in0=rw1_sb[:, dc, :],
                                               scalar=pooled[:, dc:dc + 1], in1=hacc,
                                               op0=mybir.AluOpType.mult,
                                               op1=mybir.AluOpType.add)
            onesf = const.tile([P, 1], F32, name="onesf")
            nc.vector.memset(onesf, 1.0)
            ph = ps.tile([1, H], F32, name="ph", tag="ps")
            nc.tensor.matmul(ph, onesf, hacc, start=True, stop=True)
            hrow_pre = const.tile([1, H], F32, name="hrow_pre")
            nc.vector.scalar_tensor_tensor(out=hrow_pre, in0=ph,
                                           scalar=1.0 / S,
                                           in1=rb1_row, op0=mybir.AluOpType.mult,
                                           op1=mybir.AluOpType.add)
            h_row = const.tile([1, H], F32, name="h_row")
            nc.scalar.activation(out=h_row, in_=hrow_pre,
                                 func=mybir.ActivationFunctionType.Tanh)

            emit_transposes([2])

            logits = const.tile([1, E], F32, name="logits")
            lscr = const.tile([1, H], F32, name="lscr")
            lsum = const.tile([1, E], F32, name="lsum")
            for e in range(E):
                nc.vector.tensor_mul(lscr, h_row, rw2T_sb[:, e, :])
                nc.vector.tensor_reduce(out=lsum[:, e:e + 1], in_=lscr,
                                        axis=mybir.AxisListType.X,
                                        op=mybir.AluOpType.add)
            nc.vector.tensor_add(logits, lsum, rb2_sb)

            # dropped expert straight from logits (argmin; softmax is monotone)
            lmin = const.tile([1, 1], F32, name="lmin")
            nc.vector.tensor_reduce(out=lmin, in_=logits, axis=mybir.AxisListType.X,
                                    op=mybir.AluOpType.min)
            iota4 = const.tile([1, E], F32, name="iota4")
            for e in range(E):
                nc.vector.memset(iota4[:, e:e + 1], float(e))
            emul = const.tile([1, E], F32, name="emul")
            nc.vector.scalar_tensor_tensor(out=emul, in0=logits, scalar=lmin,
                                           in1=iota4, op0=mybir.AluOpType.is_le,
                                           op1=mybir.AluOpType.mult)
            dminf = const.tile([1, 1], F32, name="dminf")
            nc.vector.tensor_reduce(out=dminf, in_=emul, axis=mybir.AxisListType.X,
                                    op=mybir.AluOpType.add)
            iota3 = const.tile([1, K], F32, name="iota3")
            for k in range(K):
                nc.vector.memset(iota3[:, k:k + 1], float(k))
            gemask = const.tile([1, K], F32, name="gemask")
            nc.vector.tensor_scalar(out=gemask, in0=iota3, scalar1=dminf, scalar2=None,
                                    op0=mybir.AluOpType.is_ge)
            ekf = const.tile([1, K], F32, name="ekf")
            nc.vector.tensor_add(ekf, iota3, gemask)
            ekP_f = const.tile([1, K], F32, name="ekP_f")
            nc.vector.tensor_scalar(out=ekP_f, in0=ekf, scalar1=float(P), scalar2=None,
                                    op0=mybir.AluOpType.mult)
            ekP_i = const.tile([1, K], I32, name="ekP_i")
            nc.vector.tensor_copy(out=ekP_i, in_=ekP_f)

            emit_transposes([3])

            hid = xb.tile([P, FC, S], FP8, name="hid")
            # accumulate in place over x_all (x is dead after the fp16 cast)
            zacc = [x_all[:, t, :] for t in range(TC)]
            wbc3 = const.tile([P, 1, K], F32, name="wbc3")
            wbc = wbc3[:, 0, :]

            # ---------- combine weights (softmax of the same logits) ----------
            # emitted as a closure, interleaved into slot-0 GEMM1 so it never
            # delays the first weight DMAs; wc only gates the first eviction.
            def emit_wc():
                mx = const.tile([1, 1], F32, name="mx")
                nc.vector.tensor_reduce(out=mx, in_=logits,
                                        axis=mybir.AxisListType.X,
                                        op=mybir.AluOpType.max)
                sh = const.tile([1, E], F32, name="sh")
                nc.vector.tensor_scalar(out=sh, in0=logits, scalar1=mx,
                                        scalar2=None, op0=mybir.AluOpType.subtract)
                ex = const.tile([1, E], F32, name="ex")
                nc.scalar.activation(out=ex, in_=sh,
                                     func=mybir.ActivationFunctionType.Exp)
                sm = const.tile([1, 1], F32, name="sm")
                nc.vector.tensor_reduce(out=sm, in_=ex, axis=mybir.AxisListType.X,
                                        op=mybir.AluOpType.add)
                rs = const.tile([1, 1], F32, name="rs")
                nc.vector.reciprocal(out=rs, in_=sm)
                probs = const.tile([1, E], F32, name="probs")
                nc.vector.tensor_scalar(out=probs, in0=ex, scalar1=rs, scalar2=None,
                                        op0=mybir.AluOpType.mult)
                pmin = const.tile([1, 1], F32, name="pmin")
                nc.vector.tensor_reduce(out=pmin, in_=probs,
                                        axis=mybir.AxisListType.X,
                                        op=mybir.AluOpType.min)
                onec = const.tile([1, 1], F32, name="onec")
                nc.vector.memset(onec, 1.0)
                den = const.tile([1, 1], F32, name="den")
                nc.vector.tensor_sub(den, onec, pmin)
                rden = const.tile([1, 1], F32, name="rden")
                nc.vector.reciprocal(out=rden, in_=den)
                gtmask = const.tile([1, E], F32, name="gtmask")
                nc.vector.tensor_scalar(out=gtmask, in0=probs, scalar1=pmin,
                                        scalar2=None, op0=mybir.AluOpType.is_gt)
                wall = const.tile([1, E], F32, name="wall")
                nc.vector.tensor_mul(wall, probs, gtmask)
                w_sb = const.tile([1, E], F32, name="w_sb")
                nc.vector.tensor_scalar(out=w_sb, in0=wall, scalar1=rden,
                                        scalar2=None, op0=mybir.AluOpType.mult)
                wdiff = const.tile([1, K], F32, name="wdiff")
                nc.vector.tensor_sub(wdiff, w_sb[:, 1:E], w_sb[:, 0:K])
                wstep = const.tile([1, K], F32, name="wstep")
                nc.vector.tensor_mul(wstep, wdiff, gemask)
                wc = const.tile([1, K], F32, name="wc")
                nc.vector.tensor_add(wc, w_sb[:, 0:K], wstep)
                # fold the 1/SW2 weight descale into the combine coefficients
                wcs = const.tile([1, K], F32, name="wcs")
                nc.vector.tensor_scalar(out=wcs, in0=wc, scalar1=1.0 / SW2,
                                        scalar2=None, op0=mybir.AluOpType.mult)
                nc.gpsimd.partition_broadcast(wbc3[:, 0, :], wcs, channels=P)
                # zb2sum and residual init
                zb2sum = const.tile([1, D], F32, name="zb2sum")
                nc.vector.tensor_scalar(out=zb2sum, in0=zb2_sb[:, 0, :],
                                        scalar1=w_sb[:, 0:1], scalar2=None,
                                        op0=mybir.AluOpType.mult)
                for e in range(1, E):
                    nc.vector.scalar_tensor_tensor(out=zb2sum, in0=zb2_sb[:, e, :],
                                                   scalar=w_sb[:, e:e + 1],
                                                   in1=zb2sum,
                                                   op0=mybir.AluOpType.mult,
                                                   op1=mybir.AluOpType.add)
                zb2b3 = const.tile([P, 1, D], F32, name="zb2b3")
                nc.gpsimd.partition_broadcast(zb2b3[:, 0, :], zb2sum, channels=P)
                for t in range(TC):
                    nc.vector.tensor_add(zacc[t], zacc[t], zb2b3[:, 0, :])

            # ---------- 3 expert slots, runtime-indexed weight streams ----------
            # zw1 streams on the scalar ring (with the GELUs, as in the
            # baseline schedule that sustains peak matmul rate); zw2 on sync.
            for k in range(K):
                rP_sy = nc.sync.alloc_register(f"rP_sy{k}")
                nc.reg_load(rP_sy, ekP_i[:, k:k + 1])
                ekP_s = nc.sync.snap(rP_sy)
                rP_sc = nc.scalar.alloc_register(f"rP_sc{k}")
                nc.reg_load(rP_sc, ekP_i[:, k:k + 1])
                ekP_c = nc.scalar.snap(rP_sc)

                zb1_sb = wstream.tile([P, FC], F32, name=f"zb1_sb{k}", tag="zb1",
                                      bufs=2)
                nc.scalar.dma_start(out=zb1_sb, in_=zb1_d[bass.ds(ekP_c, P), :])

                # ---------- GEMM1: fp8 DR pairs + fp16 chunks per fc ----------
                N8 = 4       # fp8 zw1 chunks (8 fc each)
                NH16 = 16    # fp16 zw1 chunks (2 fc each)
                PREH = 6     # fp16 chunks prefetched ahead
                zw1q8 = [None] * N8
                zw1h = [None] * NH16

                def load_q8(c, kk=k, ekc=ekP_c):
                    t8 = wstream.tile([P, 8, DRP, 2, P], FP8, name=f"zw1q{kk}_{c}",
                                      tag="zw1q8", bufs=N8)
                    nc.scalar.dma_start(out=t8,
                                        in_=zw1q_d[bass.ds(ekc, P), 8 * c:8 * c + 8,
                                                   :, :, :])
                    zw1q8[c] = t8

                def load_h(c, kk=k, ekc=ekP_c):
                    th = wstream.tile([P, 2, BF_DC, P], F16, name=f"zw1h{kk}_{c}",
                                      tag="zw1h", bufs=PREH + 2)
                    nc.scalar.dma_start(out=th,
                                        in_=zw1h_d[bass.ds(ekc, P), 2 * c:2 * c + 2,
                                                   :, :])
                    zw1h[c] = th

                if DRP:
                    load_q8(0)
                    load_q8(1)
                for c in range(PREH):
                    load_h(c)
                for fc in range(FC):
                    if fc % 8 == 0 and DRP and fc // 8 + 2 < N8:
                        load_q8(fc // 8 + 2)
                    if fc % 2 == 0 and fc // 2 + PREH < NH16:
                        load_h(fc // 2 + PREH)
                    p1 = ps.tile([P, S], F32, name=f"p1_{k}_{fc}", tag="ps")
                    for dp in range(DRP):
                        nc.tensor.matmul(p1, zw1q8[fc // 8][:, fc % 8, dp, :, :],
                                         xT8[dp], start=(dp == 0), stop=False,
                                         perf_mode=DR)
                    for j in range(BF_DC):
                        nc.tensor.matmul(p1, zw1h[fc // 2][:, fc % 2, j, :],
                                         xT16[j], start=(DRP == 0 and j == 0),
                                         stop=(j == BF_DC - 1))
                    nc.scalar.activation(out=hid[:, fc, :], in_=p1, func=GELU,
                                         bias=zb1_sb[:, fc:fc + 1],
                                         scale=1.0 / (SX * SW1))
                    if k == 0 and fc == 2:
                        emit_wc()

                # ---------- GEMM2 (fp8 DoubleRow over 16 f-pairs, fp-major) ----
                p2 = []
                for t in range(TC):
                    for nh in range(NH):
                        p2t = ps.tile([P, 512], F32, name=f"p2_{k}_{t}_{nh}",
                                      tag="ps")
                        p2.append(p2t)
                PRE2 = 5
                zw2q = [None] * FP2

                def load_zw2(fp, kk=k, eks=ekP_s):
                    t8 = wstream.tile([P, 2, D], FP8, name=f"zw2q{kk}_{fp}",
                                      tag="zw2q", bufs=PRE2 + 2)
                    nc.sync.dma_start(
                        out=t8,
                        in_=zw2_d[bass.ds(eks, P), fp, :, :])
                    zw2q[fp] = t8

                for fp in range(PRE2):
                    load_zw2(fp)
                for fp in range(FP2):
                    if fp + PRE2 < FP2:
                        load_zw2(fp + PRE2)
                    for t in range(TC):
                        for nh in range(NH):
                            nc.tensor.matmul(
                                p2[t * NH + nh],
                                hid[:, 2 * fp:2 * fp + 2, t * P:(t + 1) * P],
                                zw2q[fp][:, :, nh * 512:(nh + 1) * 512],
                                start=(fp == 0), stop=(fp == FP2 - 1),
                                perf_mode=DR)
                for t in range(TC):
                    for nh in range(NH):
                        sl = slice(nh * 512, (nh + 1) * 512)
                        nc.vector.scalar_tensor_tensor(
                            out=zacc[t][:, sl], in0=p2[t * NH + nh],
                            scalar=wbc[:, k:k + 1], in1=zacc[t][:, sl],
                            op0=mybir.AluOpType.mult, op1=mybir.AluOpType.add)
                        if k == K - 1:
                            eng = nc.scalar if (t * NH + nh) % 2 == 0 else nc.sync
                            eng.dma_start(out=out_d[t * P:(t + 1) * P, sl],
                                          in_=zacc[t][:, sl])

    nc.finalize()
    return nc


_NC_CACHE = None


def _get_nc():
    global _NC_CACHE
    if _NC_CACHE is None:
        _NC_CACHE = build_nc()
    return _NC_CACHE


def kernel(x, rw1, rb1, rw2, rb2, zw1, zb1, zw2, zb2, **run_kwargs):
    x = np.ascontiguousarray(np.asarray(x, dtype=np.float32))
    zw1 = np.asarray(zw1, np.float32)
    zw2 = np.asarray(zw2, np.float32)
    zb1 = np.asarray(zb1, np.float32)
    FP8NP = ml_dtypes.float8_e4m3
    shared = {
        "rw1": np.ascontiguousarray(np.asarray(rw1, np.float32)),
        "rb1": np.ascontiguousarray(np.asarray(rb1, np.float32)),
        "rw2": np.ascontiguousarray(np.asarray(rw2, np.float32)),
        "rb2": np.ascontiguousarray(np.asarray(rb2, np.float32)),
        "zb1r": np.ascontiguousarray(
            zb1.reshape(E, FC, P).transpose(0, 2, 1).reshape(E * P, FC)),
        "zb2": np.ascontiguousarray(np.asarray(zb2, np.float32)),
    }
    if DRP:
        # zw1q[e*P+p, fc, dp, i, fw] = fp8(SW1 * zw1[e, dp*256+i*128+p, fc*128+fw])
        shared["zw1q"] = np.ascontiguousarray(
            (zw1[:, :DSPLIT, :] * SW1).reshape(E, DRP, 2, P, FC, P)
            .transpose(0, 3, 4, 1, 2, 5)
            .reshape(E * P, FC, DRP, 2, P).astype(FP8NP))
    if BF_DC:
        # zw1h[e*P+p, fc, j, fw] = f16(SX*SW1 * zw1[e, DSPLIT+j*128+p, fc*128+fw])
        shared["zw1h"] = np.ascontiguousarray(
            (zw1[:, DSPLIT:, :] * (SX * SW1)).reshape(E, BF_DC, P, FC, P)
            .transpose(0, 2, 3, 1, 4)
            .reshape(E * P, FC, BF_DC, P).astype(np.float16))
    # zw2q[e*P+p, fp, i, d] = fp8(SW2 * zw2[e, fp*256+i*128+p, d])
    shared["zw2q"] = np.ascontiguousarray(
        (zw2 * SW2).reshape(E, FP2, 2, P, D)
        .transpose(0, 3, 1, 2, 4)
        .reshape(E * P, FP2, 2, D).astype(FP8NP))
    B = x.shape[0]
    nc = _get_nc()
    in_maps = [dict(shared, x=x[b]) for b in range(B)]
    res = run_bass_kernel_spmd(nc, in_maps, core_ids=list(range(B)), **run_kwargs)
    out = np.stack([res.results[b]["out"] for b in range(B)], axis=0)
    if run_kwargs:
        kernel.last_results = res
    return out


if __name__ == "__main__":
    rng = np.random.default_rng(0)
    inputs = {
        "x": rng.standard_normal((8, S, D)).astype(np.float32),
        "rw1": (rng.standard_normal((D, H)) / np.sqrt(D)).astype(np.float32),
        "rb1": np.zeros(H, np.float32),
        "rw2": (rng.standard_normal((H, E)) / np.sqrt(H)).astype(np.float32),
        "rb2": np.zeros(E, np.float32),
        "zw1": (rng.standard_normal((E, D, F)) / np.sqrt(D)).astype(np.float32),
        "zb1": np.zeros((E, F), np.float32),
        "zw2": (rng.standard_normal((E, F, D)) / np.sqrt(F)).astype(np.float32),
        "zb2": np.zeros((E, D), np.float32),
    }
    out = kernel(**inputs)
    print("out", out.shape, out.dtype, np.abs(out).max())
